# revision 13
# baseline (speedup 1.0000x reference)
"""Capsule-FC (dynamic routing) Trainium2 kernel, 8 NeuronCores.

Strategy: tensor-parallel over NOUT (32 output capsules -> 4 per core).
 - Projection u = x @ W.T + b: each core streams its (2048, 16384) W^T shard
   (host-pretransposed, n-order (o_local, d, i), bias folded in as an extra
   K row) through PE with f32r matmuls.  HBM-bound.
 - Routing (R+1 = 4 iterations) stays on-device. Each iteration needs the
   softmax over ALL 32 output capsules -> AllGather of exp(b) blocks
   ((4, 256) per core) across the 8 cores; everything else is local.
 - u is kept in SBUF in two layouts: U1 (batch on partitions) for the
   agreement update, U_A (input-capsule on partitions) for the weighted sum.
"""

import sys

sys.path.insert(0, "/opt/trn_rl_repo")

import numpy as np

import concourse.bass as bass
import concourse.mybir as mybir
import concourse.tile as tile
from concourse import bacc
import concourse.bass_utils as bass_utils
from concourse.masks import make_identity

B, NIN, DIN, NOUT, DOUT, R = 64, 256, 8, 32, 16, 3
K = NIN * DIN            # 2048 contraction dim
NC = 8                   # cores
G = NOUT // NC           # 4 output capsules per core
NL = G * DOUT * NIN      # 16384 output features per core, order (o, d, i)
KCH = 17                 # k-chunks of 128 (incl. bias row chunk)
KP = KCH * 128           # 2176 padded contraction dim
F32 = mybir.dt.float32
F32R = mybir.dt.float32r

_cache = {}


def tc_tile(tc, shape, dtype, name):
    t, _free = tc.tile(shape, dtype, name=name)
    return t



USE_F32R = False


def _r(ap):
    return ap.bitcast(F32R) if USE_F32R else ap


def _build():
    if "nc" in _cache:
        return _cache["nc"]

    nc = bacc.Bacc("TRN2", target_bir_lowering=False, debug=False,
                   enable_asserts=True, num_devices=NC)

    xT_d = nc.dram_tensor("xT", (KP, B), F32, kind="ExternalInput").ap()
    wT_d = nc.dram_tensor("wT", (KP, NL), F32, kind="ExternalInput").ap()
    out_d = nc.dram_tensor("out", (G, DOUT * B), F32, kind="ExternalOutput").ap()
    DEBUG = _cache.get("debug", False)
    if DEBUG:
        u1_d = nc.dram_tensor("u1dbg", (B, NL), F32, kind="ExternalOutput").ap()
        ua0_d = nc.dram_tensor("ua0dbg", (128, G * DOUT * B), F32, kind="ExternalOutput").ap()
        ua1_d = nc.dram_tensor("ua1dbg", (128, G * DOUT * B), F32, kind="ExternalOutput").ap()
        s0_d = nc.dram_tensor("s0dbg", (G, DOUT * B), F32, kind="ExternalOutput").ap()
        b0_d = nc.dram_tensor("b0dbg", (128, 2 * G), F32, kind="ExternalOutput").ap()
        br0_d = nc.dram_tensor("br0dbg", (128, 2 * G), F32, kind="ExternalOutput").ap()
        expb_d = nc.dram_tensor("expbdbg", (G, NIN), F32, kind="ExternalOutput").ap()
        expf_d = nc.dram_tensor("expfdbg", (NOUT, NIN), F32, kind="ExternalOutput").ap()
        cds_d = nc.dram_tensor("cdsdbg", (128, 2 * G * G), F32, kind="ExternalOutput").ap()
        s1_d = nc.dram_tensor("s1dbg", (G, DOUT * B), F32, kind="ExternalOutput").ap()

    with tile.TileContext(nc) as tc, \
         tc.tile_pool(name="persist", bufs=1) as pers:
        # ---- persistent tiles ----
        u1 = pers.tile([128, NL], F32, name="u1")          # (b | pad, (o,d,i))
        ua0 = pers.tile([128, G * DOUT * B], F32, name="ua0")  # (i half 0, (o,d,b))
        ua1 = pers.tile([128, G * DOUT * B], F32, name="ua1")  # (i half 1, (o,d,b))
        ua = [ua0, ua1]
        xsb = pers.tile([128, KCH, B], F32, name="xsb")
        ident = pers.tile([128, 128], F32, name="ident")
        b20 = pers.tile([128, G], F32, name="b20")   # logits (i half 0, o)
        b21 = pers.tile([128, G], F32, name="b21")   # logits (i half 1, o)
        b2 = [b20, b21]
        sT = pers.tile([128, DOUT * G], F32, name="sT")    # (b | pad, (d,o)) squashed s
        cd0 = pers.tile([128, G * G], F32, name="cd0")     # block-diag c, iter 0
        cdA = pers.tile([128, G * G], F32, name="cdA")     # block-diag c, i-half 0
        cdB = pers.tile([128, G * G], F32, name="cdB")
        cds = [cdA, cdB]

        make_identity(nc, ident[:])
        nc.vector.memset(u1[B:128, :], 0.0)
        nc.vector.memset(sT[B:128, :], 0.0)
        nc.vector.memset(b20[:], 0.0)
        nc.vector.memset(b21[:], 0.0)
        nc.vector.memset(cdA[:], 0.0)
        nc.vector.memset(cdB[:], 0.0)
        nc.vector.memset(cd0[:], 0.0)
        for o in range(G):
            nc.vector.memset(cd0[:, o * G + o : o * G + o + 1], 1.0 / NOUT)

        nc.sync.dma_start(xsb[:], xT_d.rearrange("(kc p) b -> p kc b", p=128))

        # ---- phase 1: projection ----
        with tc.tile_pool(name="wpool", bufs=12) as wpool, \
             tc.tile_pool(name="pp", bufs=4, space="PSUM") as pp, \
             tc.tile_pool(name="ptp", bufs=4, space="PSUM") as ptp:
            for js in range(16):              # 1024-wide n superchunks
                ps = [pp.tile([B, 512], F32, tag="ps", name=f"ps{js}_{jj}") for jj in range(2)]
                for kc in range(KCH):
                    wt = wpool.tile([128, 1024], F32, tag="w")
                    nc.sync.dma_start(
                        wt[:], wT_d[kc * 128:(kc + 1) * 128,
                                    js * 1024:(js + 1) * 1024])
                    for jj in range(2):
                        nc.tensor.matmul(
                            ps[jj][:], _r(xsb[:, kc, :]),
                            _r(wt[:, jj * 512:(jj + 1) * 512]),
                            start=(kc == 0), stop=(kc == KCH - 1))
                for jj in range(2):
                    j = js * 2 + jj           # 512-wide n chunk index
                    o, dp = j // 8, (j % 8) * 2
                    nc.vector.tensor_copy(
                        out=u1[0:B, j * 512:(j + 1) * 512], in_=ps[jj][:])
                    for t in range(4):        # transpose 128-col blocks
                        d, ih = dp + t // 2, t % 2
                        pt = ptp.tile([128, B], F32, tag="pt")
                        nc.tensor.transpose(
                            pt[:],
                            u1[0:B, j * 512 + t * 128: j * 512 + (t + 1) * 128],
                            ident[:B, :B])
                        nc.vector.tensor_copy(
                            out=ua[ih][:, (o * DOUT + d) * B:(o * DOUT + d + 1) * B],
                            in_=pt[:])

        if DEBUG:
            nc.sync.dma_start(u1_d[:], u1[0:B, :])
            nc.sync.dma_start(ua0_d[:], ua0[:])
            nc.sync.dma_start(ua1_d[:], ua1[:])

        # ---- phase 2: routing ----
        u1v = u1[:].rearrange("p (o d i) -> p o d i", o=G, d=DOUT, i=NIN)
        with tc.tile_pool(name="rsb", bufs=2) as rsb, \
             tc.tile_pool(name="rps", bufs=2, space="PSUM") as rps, \
             tc.tile_pool(name="rpt", bufs=3, space="PSUM") as rpt, \
             tc.tile_pool(name="dram", bufs=2, space="DRAM") as dram:
            for r in range(R + 1):
                if r == 0:
                    csrc = [cd0, cd0]
                else:
                    # stable softmax over all NOUT: AllGather raw b, then
                    # exp(b - rowmax) locally (ACT bias is per-partition)
                    bg = rsb.tile([G, NIN], F32, tag="bg")
                    for ih in range(2):
                        pe_ = rpt.tile([G, 128], F32, tag="tp", name=f"pe_{r}_{ih}")
                        nc.tensor.transpose(pe_[:], b2[ih][:], ident[:])
                        nc.vector.tensor_copy(
                            out=bg[:, ih * 128:(ih + 1) * 128], in_=pe_[:])
                    cc_in = dram.tile([G, NIN], F32, tag="cci")
                    cc_out = dram.tile([NOUT, NIN], F32, tag="cco",
                                       addr_space="Shared")
                    nc.sync.dma_start(cc_in[:], bg[:])
                    nc.gpsimd.collective_compute(
                        "AllGather", mybir.AluOpType.bypass,
                        replica_groups=[list(range(NC))],
                        ins=[cc_in[:]], outs=[cc_out[:]])
                    bfull = rsb.tile([NOUT, NIN], F32, tag="bfull")
                    nc.sync.dma_start(bfull[:], cc_out[:])
                    if DEBUG and r == 1:
                        nc.sync.dma_start(expb_d[:], bg[:])
                        nc.sync.dma_start(expf_d[:], bfull[:])
                    for ih in range(2):
                        pf = rpt.tile([128, NOUT], F32, tag="tp", name=f"pf_{r}_{ih}")
                        nc.tensor.transpose(
                            pf[:], bfull[:, ih * 128:(ih + 1) * 128],
                            ident[:NOUT, :NOUT])
                        m = rsb.tile([128, 1], F32, tag="m")
                        nc.vector.reduce_max(m[:], pf[:],
                                             axis=mybir.AxisListType.X)
                        negm = rsb.tile([128, 1], F32, tag="negm")
                        nc.vector.tensor_scalar_mul(negm[:], m[:], -1.0)
                        eT = rsb.tile([128, NOUT], F32, tag="eT")
                        nc.scalar.activation(eT[:], pf[:],
                                             mybir.ActivationFunctionType.Exp,
                                             bias=negm[:])
                        den = rsb.tile([128, 1], F32, tag="den")
                        nc.vector.reduce_sum(den[:], eT[:],
                                             axis=mybir.AxisListType.X)
                        rec = rsb.tile([128, 1], F32, tag="rec")
                        nc.vector.reciprocal(rec[:], den[:])
                        eo = rsb.tile([128, G], F32, tag="eo")
                        nc.scalar.activation(eo[:], b2[ih][:],
                                             mybir.ActivationFunctionType.Exp,
                                             bias=negm[:])
                        for o in range(G):
                            nc.vector.tensor_scalar_mul(
                                cds[ih][:, o * G + o: o * G + o + 1],
                                eo[:, o:o + 1], rec[:])
                    csrc = cds
                    if DEBUG and r == 1:
                        nc.sync.dma_start(cds_d[:, 0:G * G], cdA[:])
                        nc.sync.dma_start(cds_d[:, G * G:2 * G * G], cdB[:])

                # s[o, (d,b)] = sum_i c[i,o] u[i, (o,d,b)]
                s_ps = rps.tile([G, DOUT * B], F32, tag="big")
                for h in range(2):
                    for ih in range(2):
                        for o in range(G):
                            nc.tensor.matmul(
                                s_ps[:, h * 512:(h + 1) * 512],
                                _r(csrc[ih][:, o * G:(o + 1) * G]),
                                _r(ua[ih][:, o * DOUT * B + h * 512:
                                          o * DOUT * B + (h + 1) * 512]),
                                start=(ih == 0 and o == 0),
                                stop=(ih == 1 and o == G - 1))

                # squash over d  (free layout is (d, b))
                s_raw = rsb.tile([G, DOUT * B], F32, tag="sraw")
                nc.vector.tensor_copy(out=s_raw[:], in_=s_ps[:])
                sq = rsb.tile([G, DOUT * B], F32, tag="sq")
                nc.scalar.square(sq[:], s_ps[:])
                nrm2 = rsb.tile([G, B], F32, tag="nrm2")
                nc.vector.reduce_sum(
                    nrm2[:], sq[:].rearrange("p (d b) -> p b d", d=DOUT, b=B),
                    axis=mybir.AxisListType.X)
                l2 = rsb.tile([G, B], F32, tag="l2")
                nc.scalar.sqrt(l2[:], nrm2[:])
                den1 = rsb.tile([G, B], F32, tag="den1")
                nc.vector.tensor_scalar_add(den1[:], nrm2[:], 1.0)
                rden = rsb.tile([G, B], F32, tag="rden")
                nc.vector.reciprocal(rden[:], den1[:])
                scl = rsb.tile([G, B], F32, tag="scl")
                nc.vector.tensor_tensor(scl[:], l2[:], rden[:],
                                        op=mybir.AluOpType.mult)
                s_sq = rsb.tile([G, DOUT * B], F32, tag="ssq")
                nc.vector.tensor_tensor(
                    s_sq[:].rearrange("p (d b) -> p b d", d=DOUT, b=B),
                    s_raw[:].rearrange("p (d b) -> p b d", d=DOUT, b=B),
                    scl[:, :, None].to_broadcast((G, B, DOUT)),
                    op=mybir.AluOpType.mult)

                if DEBUG and r == 0:
                    nc.sync.dma_start(s0_d[:], s_sq[:])
                if DEBUG and r == 1:
                    nc.sync.dma_start(s1_d[:], s_sq[:])
                if r < R:
                    # sT[(b), (d,o)] = s_sq[o, (d,b)] transposed per d-block
                    for d in range(DOUT):
                        pst = rpt.tile([B, G], F32, tag="tp")
                        nc.tensor.transpose(
                            pst[:], s_sq[:, d * B:(d + 1) * B], ident[:G, :G])
                        nc.vector.tensor_copy(
                            out=sT[0:B, d * G:(d + 1) * G], in_=pst[:])
                    # delta_b[o', (o,i)] = sum_(d,b) sT[b, (d,o')] u1[b, (o,d,i)]
                    b_ps = rps.tile([G, G * NIN], F32, tag="big")
                    for h in range(2):
                        for d in range(DOUT):
                            nc.tensor.matmul(
                                b_ps[:, h * 512:(h + 1) * 512],
                                _r(sT[:, d * G:(d + 1) * G]),
                                _r(u1v[:, 2 * h:2 * h + 2, d, :]),
                                start=(d == 0), stop=(d == DOUT - 1))
                    bp_sb = rsb.tile([G, G * NIN], F32, tag="bpsb")
                    nc.vector.tensor_copy(out=bp_sb[:], in_=b_ps[:])
                    for ih in range(2):
                        for o in range(G):
                            pbt = rpt.tile([128, G], F32, tag="tp",
                                           name=f"pbt_{r}_{ih}_{o}")
                            nc.tensor.transpose(
                                pbt[:],
                                bp_sb[:, o * NIN + ih * 128:
                                      o * NIN + (ih + 1) * 128],
                                ident[:G, :G])
                            nc.vector.tensor_tensor(
                                b2[ih][:, o:o + 1], b2[ih][:, o:o + 1],
                                pbt[:, o:o + 1], op=mybir.AluOpType.add)
                    if DEBUG and r == 0:
                        nc.sync.dma_start(br0_d[:, 0:G], b2[0][:])
                        nc.sync.dma_start(br0_d[:, G:2 * G], b2[1][:])
                else:
                    nc.sync.dma_start(out_d[:], s_sq[:])
            if DEBUG:
                nc.sync.dma_start(b0_d[:, 0:G], b2[0][:])
                nc.sync.dma_start(b0_d[:, G:2 * G], b2[1][:])

    nc.compile()
    _cache["nc"] = nc
    return nc


def kernel(x, W, b_W):
    nc = _build()
    x = np.ascontiguousarray(x, dtype=np.float32)
    W = np.ascontiguousarray(W, dtype=np.float32)
    b_W = np.ascontiguousarray(b_W, dtype=np.float32)

    xT = np.zeros((KP, B), np.float32)
    xT[:K] = x.reshape(B, K).T
    xT[K] = 1.0

    W4 = W.reshape(NIN, NOUT, DOUT, K)
    b3 = b_W.reshape(NIN, NOUT, DOUT)
    in_maps = []
    for c in range(NC):
        wT = np.zeros((KP, NL), np.float32)
        wc = W4[:, G * c:G * (c + 1)].transpose(3, 1, 2, 0)  # (K, o, d, i)
        wT[:K] = wc.reshape(K, NL)
        wT[K] = b3[:, G * c:G * (c + 1)].transpose(1, 2, 0).reshape(NL)
        in_maps.append({"xT": xT, "wT": wT})

    _cache["last_in_maps"] = in_maps
    res = bass_utils.run_bass_kernel_spmd(nc, in_maps, core_ids=list(range(NC)))
    _cache["last_result"] = res
    outs = []
    for c in range(NC):
        oc = res.results[c]["out"].reshape(G, DOUT, B).transpose(2, 0, 1)
        outs.append(oc)
    return np.concatenate(outs, axis=1).astype(np.float32)
